# revision 30
# baseline (speedup 1.0000x reference)
"""Trainium2 Bass kernel for nn_AttributePredictor (moe_routing).

Strategy (data-parallel over 8 NeuronCores, 64 boxes per core):
  - Host routes 64 boxes to each core and packs Kp=3 "strip regions"
    [128, 6*1024]: partition p<64 carries the A-half rows of box p's ROI
    rect, partition p+64 the B-half, halving the row-fold depth and using
    all 128 partitions.  Invalid columns are pre-set to -1e30.
  - Device: regions stream in over both HWDGE rings (weights queued
    behind), per-region column-max chains run as regions land, processed
    in d-halves so transposes + the FF matmul start before pooling fully
    drains.  The A/B halves are combined after a single [128,128] PE
    transpose per chunk (max of the two free-dim halves).  FF + LayerNorm
    + exact GELU + all heads run in fp32; per-label selection/bias/mask
    are O(N*9) host glue.  bf16 + region-fed fp32 dummy matmuls keep the
    PE HAM clock warm through the DMA/pooling phase.
"""

import numpy as np

import concourse.bacc as bacc
import concourse.bass as bass
import concourse.mybir as mybir
import concourse.tile as tile
from concourse import bass_utils
from concourse.masks import make_identity

F32 = mybir.dt.float32
BF16 = mybir.dt.bfloat16
MAX_OP = mybir.AluOpType.max

ID2CAT = np.array([3, 5, 2, 4, 6, 3, 7, 2, 5, 4, 8, 3, 2, 6, 5, 4, 3, 9, 2, 5],
                  dtype=np.int32)
NUM_LABELS = 20
MAX_ATT = 9
IMG = 224
GRID = 14
BS, NT, DV = 32, 196, 1024
N_CORES = 8
LN_EPS = 1e-5
NEG = np.float32(-1e30)
PCOLS = 16
NH = NUM_LABELS * MAX_ATT + 1   # 181
KD = DV // 128                  # 8 contraction chunks
N_WARM = 24                     # initial PE warmup matmuls (bf16)
N_SLOT_WARM = 4                 # fp32 PE keep-warm matmuls per landed region


def _preprocess(x, boxes):
    boxes = np.asarray(boxes, dtype=np.float32)
    x = np.asarray(x, dtype=np.float32)
    nbox = boxes.shape[0]
    assert nbox % N_CORES == 0
    npc = nbox // N_CORES
    assert 2 * npc <= 128, f"boxes/core {npc} too large for A/B split"

    scale = np.float32(GRID / IMG)
    c = np.floor(boxes[:, 1:] * scale + np.float32(0.5)).astype(np.int32)
    x1, y1, x2, y2 = c[:, 0], c[:, 1], c[:, 2], c[:, 3]
    hs = np.clip(y1, 0, GRID)
    he = np.clip(np.maximum(y2 + 1, y1 + 1), 0, GRID)
    ws = np.clip(x1, 0, GRID)
    we = np.clip(np.maximum(x2 + 1, x1 + 1), 0, GRID)
    hh = he - hs
    ww = we - ws
    assert (hh > 0).all() and (ww > 0).all(), "empty ROI rect not supported"

    K_w = int(ww.max())
    assert (ws + K_w <= PCOLS).all()
    ha = (hh + 1) // 2
    hb = hh - ha
    Kp = int(ha.max())

    xp = np.full((BS, GRID, PCOLS, DV), NEG, dtype=np.float32)
    xp[:, :, :GRID, :] = x.reshape(BS, GRID, GRID, DV)
    xflat = xp.reshape(-1)
    SW = K_w * DV
    nwin = BS * GRID * PCOLS - (K_w - 1)
    view = np.lib.stride_tricks.as_strided(
        xflat, shape=(nwin, SW), strides=(DV * 4, 4))

    bidx = boxes[:, 0].astype(np.int32)
    base = (bidx * GRID + hs) * PCOLS + ws

    # per-core box order: height-descending so region r>0 only needs a
    # prefix of A rows / B rows (exact-prefix packing)
    orders = []
    for cc in range(N_CORES):
        g = np.arange(cc * npc, (cc + 1) * npc)
        orders.append(g[np.argsort(-hh[g], kind="stable")])
    HD2 = DV // 2
    packeds = []
    for cc in range(N_CORES):
        o = orders[cc]
        blocks = []
        for r in range(Kp):
            ra = np.minimum(r, ha[o] - 1)
            rowsA = base[o] + ra * PCOLS
            rb = np.where(hb[o] > 0,
                          ha[o] + np.minimum(r, np.maximum(hb[o] - 1, 0)), 0)
            rowsB = base[o] + rb * PCOLS
            rows_ = np.concatenate([rowsA, rowsB])
            blk = view[rows_].copy().reshape(2 * npc, K_w, 2, HD2)
            for i, gg in enumerate(o):
                w = int(ww[gg])
                if w < K_w:
                    blk[i, w:, :, :] = NEG
                    blk[npc + i, w:, :, :] = NEG
            for hhalf in range(2):
                blocks.append(np.ascontiguousarray(
                    blk[:, :, hhalf, :].reshape(2 * npc, K_w * HD2)))
        packeds.append(np.concatenate(blocks, axis=0))

    cfg = dict(npc=npc, Kp=Kp, K_w=K_w)
    return cfg, packeds, orders


def _build_program(cfg, skip_bff, skip_ln_affine):
    npc, Kp, K_w = cfg["npc"], cfg["Kp"], cfg["K_w"]
    SW = K_w * DV
    P2 = 2 * npc
    HD = DV // 2        # d-half width (512)
    HW = HD             # alias
    KH = KD // 2        # k-chunks per d-half (4)

    nc = bacc.Bacc("TRN2", target_bir_lowering=False, debug=False,
                   num_devices=N_CORES)

    SWH = K_w * HD
    pk_d = nc.dram_tensor("packed", [Kp * 2 * P2, SWH], F32, kind="ExternalInput")
    wt_d = nc.dram_tensor("wt", [128, KD, DV], F32, kind="ExternalInput")
    ht_d = nc.dram_tensor("ht", [128, KD, NH], F32, kind="ExternalInput")
    if not skip_bff:
        bff_d = nc.dram_tensor("bff", [1, DV], F32, kind="ExternalInput")
    if not skip_ln_affine:
        lng_d = nc.dram_tensor("lng", [1, DV], F32, kind="ExternalInput")
        lnb_d = nc.dram_tensor("lnb", [1, DV], F32, kind="ExternalInput")
    y_d = nc.dram_tensor("y_out", [npc, DV], F32, kind="ExternalOutput")
    lg_d = nc.dram_tensor("lg_out", [npc, NH], F32, kind="ExternalOutput")

    with tile.TileContext(nc) as tc:
        with (
            tc.tile_pool(name="const", bufs=1) as const,
            tc.tile_pool(name="work", bufs=1) as work,
            tc.tile_pool(name="ps_t", bufs=3, space="PSUM") as ps_t,
            tc.tile_pool(name="ps_mm", bufs=1, space="PSUM") as ps_mm,
            tc.tile_pool(name="ps_w", bufs=1, space="PSUM") as ps_w,
        ):
            ident = const.tile([128, 128], F32)
            make_identity(nc, ident[:])
            eps_t = const.tile([npc, 1], F32)
            nc.vector.memset(eps_t[:], LN_EPS)

            # preload the Sqrt ACT table off the critical path (Gelu's
            # load later evicts it only after LN is done with it)
            tbl = const.tile([1, 1], F32)
            nc.scalar.activation(out=tbl[:], in_=eps_t[0:1, 0:1],
                                 func=mybir.ActivationFunctionType.Sqrt,
                                 bias=eps_t[0:1, :], scale=1.0)

            # ---- input DMAs: regions first (h0 on sync, h1 on scalar),
            # then the weight chunks each half's FF needs, then ht
            NPAIR = (K_w + 1) // 2
            slots = [[None] * 2 for _ in range(Kp)]
            engs = [nc.sync, nc.scalar]
            for r in range(Kp):
                for hh in range(2):
                    prs = []
                    row0 = (r * 2 + hh) * P2
                    for jp in range(NPAIR):
                        c0 = jp * 2 * HD
                        c1 = min((jp + 1) * 2 * HD, SWH)
                        pr = work.tile([P2, c1 - c0], F32,
                                       name=f"slot{r}_{hh}_{jp}",
                                       tag=f"slot{r}_{hh}_{jp}")
                        engs[hh].dma_start(pr[:],
                                           pk_d[row0:row0 + P2, c0:c1])
                        prs.append(pr)
                    slots[r][hh] = prs
            wt = const.tile([128, KD, DV], F32)
            nc.sync.dma_start(wt[:, 0:KH, :], wt_d[:, 0:KH, :])
            nc.scalar.dma_start(wt[:, KH:KD, :], wt_d[:, KH:KD, :])
            ht = const.tile([128, KD, NH], F32)
            nc.sync.dma_start(ht[:], ht_d[:])
            if not skip_bff:
                ones1 = const.tile([1, npc], F32)
                nc.vector.memset(ones1[:], 1.0)
                bff = const.tile([1, DV], F32)
                nc.scalar.dma_start(bff[:], bff_d[:])
            if not skip_ln_affine:
                gb = const.tile([npc, DV], F32, name="gb")
                nc.gpsimd.dma_start(
                    out=gb[:],
                    in_=bass.AP(tensor=lng_d, offset=0, ap=[[0, npc], [1, DV]]))
                bb = const.tile([npc, DV], F32, name="bb")
                nc.gpsimd.dma_start(
                    out=bb[:],
                    in_=bass.AP(tensor=lnb_d, offset=0, ap=[[0, npc], [1, DV]]))

            # ---- PE warmup: initial bf16 burst + keep-warm matmuls fed
            # by each landed region (dep keeps them adjacent in time)
            wp = ps_w.tile([128, 512], F32)
            if N_WARM:
                wsrc = const.tile([128, 512], BF16)
                nc.vector.memset(wsrc[:], 0.0)
                wident = const.tile([128, 128], BF16)
                nc.vector.memset(wident[:], 0.0)
                for _ in range(N_WARM):
                    nc.tensor.matmul(out=wp[:], lhsT=wident[:], rhs=wsrc[:],
                                     start=True, stop=True)
            for r in range(Kp):
                for hh in range(2):
                    for d in range(N_SLOT_WARM // 2):
                        nc.tensor.matmul(
                            out=wp[:], lhsT=ident[:],
                            rhs=slots[r][hh][d % NPAIR][:, 0:512],
                            start=True, stop=True)

            # ---- pooling + FF, pipelined over d-halves ----
            # col-max chains per (region, d-half); folds; A/B combine is
            # fused into the transposes (max of two transposed halves).
            h_ps = [ps_mm.tile([npc, HD], F32, name=f"hps{h}") for h in range(2)]
            pts = [None] * KD
            import contextlib
            for h in range(2):
                prio = tc.high_priority() if h == 0 else contextlib.nullcontext()
                ctx_prio = prio.__enter__() if hasattr(prio, "__enter__") else None
                cms = []
                for r in range(Kp):
                    prs = slots[r][h]
                    segs = [prs[j // 2][:, (j % 2) * HD:(j % 2 + 1) * HD]
                            for j in range(K_w)]
                    lvl = 0
                    while len(segs) > 1:
                        nxt = []
                        for a in range(0, len(segs) - 1, 2):
                            t = work.tile([P2, HD], F32,
                                          name=f"cm{r}_{h}_{lvl}_{a}",
                                          tag=f"cm{r}_{h}_{lvl}_{a}")
                            nc.vector.tensor_tensor(out=t[:], in0=segs[a],
                                                    in1=segs[a + 1], op=MAX_OP)
                            nxt.append(t[:])
                        if len(segs) % 2:
                            nxt.append(segs[-1])
                        segs = nxt
                        lvl += 1
                    cms.append(segs[0])
                for r in range(1, Kp):
                    nc.vector.tensor_tensor(
                        out=cms[0][:], in0=cms[0][:], in1=cms[r][:], op=MAX_OP)

                # transposes with fused A/B max; then FF k-chunks for this half
                cmf = cms[0]
                for kk in range(KH):
                    k = h * KH + kk
                    co = kk * 128
                    psA = ps_t.tile([128, P2], F32, name=f"psA{k}", tag="psA")
                    nc.tensor.transpose(
                        out=psA[:], in_=cmf[:, co:co + 128],
                        identity=ident[:])
                    pt = work.tile([128, npc], F32, name=f"pt{k}", tag=f"pt{k}")
                    nc.vector.tensor_copy(out=pt[:], in_=psA[:, 0:npc])
                    nc.vector.tensor_tensor(
                        out=pt[:], in0=pt[:],
                        in1=psA[:, npc:P2], op=MAX_OP)
                    pts[k] = pt
                for kk in range(KH):
                    k = h * KH + kk
                    for nh in range(2):
                        nc.tensor.matmul(
                            out=h_ps[nh][:], lhsT=pts[k][:],
                            rhs=wt[:, k, nh * HD:(nh + 1) * HD],
                            start=(k == 0), stop=(k == KD - 1 and skip_bff))
                prio.__exit__(None, None, None)
            if not skip_bff:
                for nh in range(2):
                    nc.tensor.matmul(
                        out=h_ps[nh][:], lhsT=ones1[0:1, :],
                        rhs=bff[0:1, nh * HD:(nh + 1) * HD],
                        start=False, stop=True)

            # ---- LayerNorm ----
            stats = work.tile([npc, 2, 6], F32)
            for s in range(2):
                nc.vector.bn_stats(out=stats[:, s, :], in_=h_ps[s][:])
            mv = work.tile([npc, 2], F32)
            nc.vector.bn_aggr(out=mv[:], in_=stats[:])
            std = work.tile([npc, 1], F32)
            nc.scalar.activation(out=std[:], in_=mv[:, 1:2],
                                 func=mybir.ActivationFunctionType.Sqrt,
                                 bias=eps_t[:], scale=1.0)
            rstd = work.tile([npc, 1], F32)
            nc.vector.reciprocal(out=rstd[:], in_=std[:])

            xn = work.tile([npc, DV], F32)
            for s in range(2):
                nc.vector.tensor_scalar(
                    out=xn[:, s * HD:(s + 1) * HD], in0=h_ps[s][:],
                    scalar1=mv[:, 0:1], scalar2=rstd[:],
                    op0=mybir.AluOpType.subtract, op1=mybir.AluOpType.mult)
            if not skip_ln_affine:
                nc.vector.tensor_mul(out=xn[:], in0=xn[:], in1=gb[:])
                nc.vector.tensor_add(out=xn[:], in0=xn[:], in1=bb[:])

            # ---- exact GELU (y_out path) ----
            y_sb = work.tile([npc, DV], F32)
            nc.scalar.activation(out=y_sb[:], in_=xn[:],
                                 func=mybir.ActivationFunctionType.Gelu)
            nc.sync.dma_start(y_d[:], y_sb[:])

            # ---- heads: transpose pre-GELU xn, gelu on transposed tiles ----
            lg_ps = ps_mm.tile([npc, NH], F32)
            for k in range(KD):
                psA = ps_t.tile([128, npc], F32, name=f"ypsA{k}", tag="psA")
                nc.tensor.transpose(
                    out=psA[:], in_=xn[:, k * 128:(k + 1) * 128],
                    identity=ident[0:npc, 0:npc])
                yt = work.tile([128, npc], F32, name=f"yt{k}", tag=f"yt{k}")
                nc.scalar.activation(out=yt[:], in_=psA[:],
                                     func=mybir.ActivationFunctionType.Gelu)
                nc.tensor.matmul(out=lg_ps[:], lhsT=yt[:], rhs=ht[:, k, :],
                                 start=(k == 0), stop=(k == KD - 1))
            lg_sb = work.tile([npc, NH], F32)
            nc.vector.tensor_copy(out=lg_sb[:], in_=lg_ps[:])
            nc.sync.dma_start(lg_d[:], lg_sb[:])

    nc.compile()
    return nc


def _weights_layout(W_ff, Wh, Wd):
    Wt = np.ascontiguousarray(W_ff.astype(np.float32).T)
    wt = np.ascontiguousarray(Wt.reshape(KD, 128, DV).transpose(1, 0, 2))
    H = np.concatenate([Wh.astype(np.float32).reshape(NUM_LABELS * MAX_ATT, DV),
                        Wd.astype(np.float32).reshape(1, DV)], axis=0)
    Ht = np.ascontiguousarray(H.T)
    ht = np.ascontiguousarray(Ht.reshape(KD, 128, NH).transpose(1, 0, 2))
    return wt, ht


def build_and_run(inputs, trace=False, trace_cores=None):
    x = np.asarray(inputs["x"], dtype=np.float32)
    boxes = np.asarray(inputs["boxes"], dtype=np.float32)
    box_labels = np.asarray(inputs["box_labels"], dtype=np.int32)
    W_ff = np.asarray(inputs["W_ff"], dtype=np.float32)
    b_ff = np.asarray(inputs["b_ff"], dtype=np.float32)
    ln_g = np.asarray(inputs["ln_g"], dtype=np.float32)
    ln_b = np.asarray(inputs["ln_b"], dtype=np.float32)
    Wh = np.asarray(inputs["Wh"], dtype=np.float32)
    bh = np.asarray(inputs["bh"], dtype=np.float32)
    Wd = np.asarray(inputs["Wd"], dtype=np.float32)
    bd = np.asarray(inputs["bd"], dtype=np.float32)

    skip_bff = bool(np.all(b_ff == 0.0))
    skip_ln_affine = bool(np.all(ln_g == 1.0) and np.all(ln_b == 0.0))

    cfg, packeds, orders = _preprocess(x, boxes)
    npc = cfg["npc"]
    nc = _build_program(cfg, skip_bff, skip_ln_affine)
    wt, ht = _weights_layout(W_ff, Wh, Wd)

    in_maps = []
    for cc in range(N_CORES):
        m = {"packed": packeds[cc], "wt": wt, "ht": ht}
        if not skip_bff:
            m["bff"] = np.ascontiguousarray(b_ff.reshape(1, DV))
        if not skip_ln_affine:
            m["lng"] = np.ascontiguousarray(ln_g.reshape(1, DV))
            m["lnb"] = np.ascontiguousarray(ln_b.reshape(1, DV))
        in_maps.append(m)

    kw = {}
    if trace:
        kw = dict(trace=True)
        if trace_cores is not None:
            kw["trace_cores"] = trace_cores
    res = bass_utils.run_bass_kernel_spmd(
        nc, in_maps, core_ids=list(range(N_CORES)), **kw)

    nbox = boxes.shape[0]
    y_full = np.zeros((nbox, DV), np.float32)
    logits_all = np.zeros((nbox, NH), np.float32)
    for cc in range(N_CORES):
        y_full[orders[cc]] = res.results[cc]["y_out"][:npc]
        logits_all[orders[cc]] = res.results[cc]["lg_out"][:npc]

    disr = logits_all[:, NH - 1:NH] + bd.reshape(1, 1)
    att = logits_all[:, :NUM_LABELS * MAX_ATT].reshape(
        nbox, NUM_LABELS, MAX_ATT)[np.arange(nbox), box_labels] + bh[box_labels]
    valid = np.arange(MAX_ATT)[None, :] < ID2CAT[box_labels][:, None]
    att = np.where(valid, att, np.float32(0.0)).astype(np.float32)
    return (y_full.astype(np.float32), att, disr.astype(np.float32)), res


def kernel(**inputs):
    boxes = np.asarray(inputs["boxes"], dtype=np.float32)
    nbox = boxes.shape[0]
    pad = (-nbox) % N_CORES
    if pad:
        inputs = dict(inputs)
        inputs["boxes"] = np.concatenate([boxes, np.tile(boxes[:1], (pad, 1))])
        inputs["box_labels"] = np.concatenate(
            [np.asarray(inputs["box_labels"], np.int32),
             np.zeros(pad, np.int32)])
    outs, _ = build_and_run(inputs, trace=False)
    if pad:
        outs = tuple(o[:nbox] for o in outs)
    return outs


# revision 31
# speedup vs baseline: 1.0768x; 1.0768x over previous
"""Trainium2 Bass kernel for nn_AttributePredictor (moe_routing).

Strategy (data-parallel over 8 NeuronCores, 64 boxes per core):
  - Host routes 64 boxes to each core and packs Kp=3 "strip regions"
    [128, 6*1024]: partition p<64 carries the A-half rows of box p's ROI
    rect, partition p+64 the B-half, halving the row-fold depth and using
    all 128 partitions.  Invalid columns are pre-set to -1e30.
  - Device: regions stream in over both HWDGE rings (weights queued
    behind), per-region column-max chains run as regions land, processed
    in d-halves so transposes + the FF matmul start before pooling fully
    drains.  The A/B halves are combined after a single [128,128] PE
    transpose per chunk (max of the two free-dim halves).  FF + LayerNorm
    + exact GELU + all heads run in fp32; per-label selection/bias/mask
    are O(N*9) host glue.  bf16 + region-fed fp32 dummy matmuls keep the
    PE HAM clock warm through the DMA/pooling phase.
"""

import numpy as np

import concourse.bacc as bacc
import concourse.bass as bass
import concourse.mybir as mybir
import concourse.tile as tile
from concourse import bass_utils
from concourse.masks import make_identity

F32 = mybir.dt.float32
BF16 = mybir.dt.bfloat16
MAX_OP = mybir.AluOpType.max

ID2CAT = np.array([3, 5, 2, 4, 6, 3, 7, 2, 5, 4, 8, 3, 2, 6, 5, 4, 3, 9, 2, 5],
                  dtype=np.int32)
NUM_LABELS = 20
MAX_ATT = 9
IMG = 224
GRID = 14
BS, NT, DV = 32, 196, 1024
N_CORES = 8
LN_EPS = 1e-5
NEG = np.float32(-1e30)
PCOLS = 16
NH = NUM_LABELS * MAX_ATT + 1   # 181
KD = DV // 128                  # 8 contraction chunks
N_WARM = 24                     # initial PE warmup matmuls (bf16)
N_SLOT_WARM = 4                 # fp32 PE keep-warm matmuls per landed region


def _preprocess(x, boxes):
    boxes = np.asarray(boxes, dtype=np.float32)
    x = np.asarray(x, dtype=np.float32)
    nbox = boxes.shape[0]
    assert nbox % N_CORES == 0
    npc = nbox // N_CORES
    assert 2 * npc <= 128, f"boxes/core {npc} too large for A/B split"

    scale = np.float32(GRID / IMG)
    c = np.floor(boxes[:, 1:] * scale + np.float32(0.5)).astype(np.int32)
    x1, y1, x2, y2 = c[:, 0], c[:, 1], c[:, 2], c[:, 3]
    hs = np.clip(y1, 0, GRID)
    he = np.clip(np.maximum(y2 + 1, y1 + 1), 0, GRID)
    ws = np.clip(x1, 0, GRID)
    we = np.clip(np.maximum(x2 + 1, x1 + 1), 0, GRID)
    hh = he - hs
    ww = we - ws
    assert (hh > 0).all() and (ww > 0).all(), "empty ROI rect not supported"

    K_w = int(ww.max())
    assert (ws + K_w <= PCOLS).all()
    ha = (hh + 1) // 2
    hb = hh - ha
    Kp = int(ha.max())

    xp = np.full((BS, GRID, PCOLS, DV), NEG, dtype=np.float32)
    xp[:, :, :GRID, :] = x.reshape(BS, GRID, GRID, DV)
    xflat = xp.reshape(-1)
    SW = K_w * DV
    nwin = BS * GRID * PCOLS - (K_w - 1)
    view = np.lib.stride_tricks.as_strided(
        xflat, shape=(nwin, SW), strides=(DV * 4, 4))

    bidx = boxes[:, 0].astype(np.int32)
    base = (bidx * GRID + hs) * PCOLS + ws

    # per-core box order: height-descending so region r>0 only needs a
    # prefix of A rows / B rows (exact-prefix packing)
    orders = []
    for cc in range(N_CORES):
        g = np.arange(cc * npc, (cc + 1) * npc)
        orders.append(g[np.argsort(-hh[g], kind="stable")])
    HD2 = DV // 2
    packeds = []
    for cc in range(N_CORES):
        o = orders[cc]
        blocks = []
        for r in range(Kp):
            ra = np.minimum(r, ha[o] - 1)
            rowsA = base[o] + ra * PCOLS
            rb = np.where(hb[o] > 0,
                          ha[o] + np.minimum(r, np.maximum(hb[o] - 1, 0)), 0)
            rowsB = base[o] + rb * PCOLS
            rows_ = np.concatenate([rowsA, rowsB])
            blk = view[rows_].copy().reshape(2 * npc, K_w, 2, HD2)
            for i, gg in enumerate(o):
                w = int(ww[gg])
                if w < K_w:
                    blk[i, w:, :, :] = NEG
                    blk[npc + i, w:, :, :] = NEG
            for hhalf in range(2):
                blocks.append(np.ascontiguousarray(
                    blk[:, :, hhalf, :].reshape(2 * npc, K_w * HD2)))
        packeds.append(np.concatenate(blocks, axis=0))

    cfg = dict(npc=npc, Kp=Kp, K_w=K_w)
    return cfg, packeds, orders


def _build_program(cfg, skip_bff, skip_ln_affine):
    npc, Kp, K_w = cfg["npc"], cfg["Kp"], cfg["K_w"]
    SW = K_w * DV
    P2 = 2 * npc
    HD = DV // 2        # d-half width (512)
    HW = HD             # alias
    KH = KD // 2        # k-chunks per d-half (4)

    nc = bacc.Bacc("TRN2", target_bir_lowering=False, debug=False,
                   num_devices=N_CORES)

    SWH = K_w * HD
    pk_d = nc.dram_tensor("packed", [Kp * 2 * P2, SWH], F32, kind="ExternalInput")
    wt_d = nc.dram_tensor("wt", [128, KD, DV], F32, kind="ExternalInput")
    ht_d = nc.dram_tensor("ht", [128, KD, NH], F32, kind="ExternalInput")
    if not skip_bff:
        bff_d = nc.dram_tensor("bff", [1, DV], F32, kind="ExternalInput")
    if not skip_ln_affine:
        lng_d = nc.dram_tensor("lng", [1, DV], F32, kind="ExternalInput")
        lnb_d = nc.dram_tensor("lnb", [1, DV], F32, kind="ExternalInput")
    y_d = nc.dram_tensor("y_out", [npc, DV], F32, kind="ExternalOutput")
    lg_d = nc.dram_tensor("lg_out", [npc, NH], F32, kind="ExternalOutput")

    with tile.TileContext(nc) as tc:
        with (
            tc.tile_pool(name="const", bufs=1) as const,
            tc.tile_pool(name="work", bufs=1) as work,
            tc.tile_pool(name="ps_t", bufs=3, space="PSUM") as ps_t,
            tc.tile_pool(name="ps_mm", bufs=1, space="PSUM") as ps_mm,
            tc.tile_pool(name="ps_w", bufs=1, space="PSUM") as ps_w,
        ):
            ident = const.tile([128, 128], F32)
            make_identity(nc, ident[:])
            eps_t = const.tile([npc, 1], F32)
            nc.vector.memset(eps_t[:], LN_EPS)

            # preload the Sqrt ACT table off the critical path (Gelu's
            # load later evicts it only after LN is done with it)
            tbl = const.tile([1, 1], F32)
            nc.scalar.activation(out=tbl[:], in_=eps_t[0:1, 0:1],
                                 func=mybir.ActivationFunctionType.Sqrt,
                                 bias=eps_t[0:1, :], scale=1.0)

            # ---- input DMAs: regions first (h0 on sync, h1 on scalar),
            # then the weight chunks each half's FF needs, then ht
            slots = [[None] * 2 for _ in range(Kp)]
            engs = [nc.sync, nc.scalar]
            for r in range(Kp):
                for hh in range(2):
                    sl = work.tile([P2, SWH], F32, name=f"slot{r}_{hh}",
                                   tag=f"slot{r}_{hh}")
                    row0 = (r * 2 + hh) * P2
                    engs[hh].dma_start(sl[:], pk_d[row0:row0 + P2, :])
                    slots[r][hh] = sl
            wt = const.tile([128, KD, DV], F32)
            nc.sync.dma_start(wt[:, 0:KH, :], wt_d[:, 0:KH, :])
            nc.scalar.dma_start(wt[:, KH:KD, :], wt_d[:, KH:KD, :])
            ht = const.tile([128, KD, NH], F32)
            nc.sync.dma_start(ht[:], ht_d[:])
            if not skip_bff:
                ones1 = const.tile([1, npc], F32)
                nc.vector.memset(ones1[:], 1.0)
                bff = const.tile([1, DV], F32)
                nc.scalar.dma_start(bff[:], bff_d[:])
            if not skip_ln_affine:
                gb = const.tile([npc, DV], F32, name="gb")
                nc.gpsimd.dma_start(
                    out=gb[:],
                    in_=bass.AP(tensor=lng_d, offset=0, ap=[[0, npc], [1, DV]]))
                bb = const.tile([npc, DV], F32, name="bb")
                nc.gpsimd.dma_start(
                    out=bb[:],
                    in_=bass.AP(tensor=lnb_d, offset=0, ap=[[0, npc], [1, DV]]))

            # ---- PE warmup: initial bf16 burst + keep-warm matmuls fed
            # by each landed region (dep keeps them adjacent in time)
            wp = ps_w.tile([128, 512], F32)
            if N_WARM:
                wsrc = const.tile([128, 512], BF16)
                nc.vector.memset(wsrc[:], 0.0)
                wident = const.tile([128, 128], BF16)
                nc.vector.memset(wident[:], 0.0)
                for _ in range(N_WARM):
                    nc.tensor.matmul(out=wp[:], lhsT=wident[:], rhs=wsrc[:],
                                     start=True, stop=True)
            for r in range(Kp):
                for hh in range(2):
                    for d in range(N_SLOT_WARM // 2):
                        nc.tensor.matmul(
                            out=wp[:], lhsT=ident[:],
                            rhs=slots[r][hh][:, d * 512:(d + 1) * 512],
                            start=True, stop=True)

            # ---- pooling + FF, pipelined over d-halves ----
            # col-max chains per (region, d-half); folds; A/B combine is
            # fused into the transposes (max of two transposed halves).
            h_ps = [ps_mm.tile([npc, HD], F32, name=f"hps{h}") for h in range(2)]
            pts = [None] * KD
            import contextlib
            for h in range(2):
                prio = tc.high_priority() if h == 0 else contextlib.nullcontext()
                ctx_prio = prio.__enter__() if hasattr(prio, "__enter__") else None
                cms = []
                for r in range(Kp):
                    sl = slots[r][h]
                    segs = [sl[:, j * HD:(j + 1) * HD] for j in range(K_w)]
                    lvl = 0
                    while len(segs) > 1:
                        nxt = []
                        for a in range(0, len(segs) - 1, 2):
                            t = work.tile([P2, HD], F32,
                                          name=f"cm{r}_{h}_{lvl}_{a}",
                                          tag=f"cm{r}_{h}_{lvl}_{a}")
                            nc.vector.tensor_tensor(out=t[:], in0=segs[a],
                                                    in1=segs[a + 1], op=MAX_OP)
                            nxt.append(t[:])
                        if len(segs) % 2:
                            nxt.append(segs[-1])
                        segs = nxt
                        lvl += 1
                    cms.append(segs[0])
                for r in range(1, Kp):
                    nc.vector.tensor_tensor(
                        out=cms[0][:], in0=cms[0][:], in1=cms[r][:], op=MAX_OP)

                # transposes with fused A/B max; then FF k-chunks for this half
                cmf = cms[0]
                for kk in range(KH):
                    k = h * KH + kk
                    co = kk * 128
                    psA = ps_t.tile([128, P2], F32, name=f"psA{k}", tag="psA")
                    nc.tensor.transpose(
                        out=psA[:], in_=cmf[:, co:co + 128],
                        identity=ident[:])
                    pt = work.tile([128, npc], F32, name=f"pt{k}", tag=f"pt{k}")
                    nc.vector.tensor_copy(out=pt[:], in_=psA[:, 0:npc])
                    nc.vector.tensor_tensor(
                        out=pt[:], in0=pt[:],
                        in1=psA[:, npc:P2], op=MAX_OP)
                    pts[k] = pt
                for kk in range(KH):
                    k = h * KH + kk
                    for nh in range(2):
                        nc.tensor.matmul(
                            out=h_ps[nh][:], lhsT=pts[k][:],
                            rhs=wt[:, k, nh * HD:(nh + 1) * HD],
                            start=(k == 0), stop=(k == KD - 1 and skip_bff))
                prio.__exit__(None, None, None)
            if not skip_bff:
                for nh in range(2):
                    nc.tensor.matmul(
                        out=h_ps[nh][:], lhsT=ones1[0:1, :],
                        rhs=bff[0:1, nh * HD:(nh + 1) * HD],
                        start=False, stop=True)

            # ---- LayerNorm ----
            stats = work.tile([npc, 2, 6], F32)
            for s in range(2):
                nc.vector.bn_stats(out=stats[:, s, :], in_=h_ps[s][:])
            mv = work.tile([npc, 2], F32)
            nc.vector.bn_aggr(out=mv[:], in_=stats[:])
            std = work.tile([npc, 1], F32)
            nc.scalar.activation(out=std[:], in_=mv[:, 1:2],
                                 func=mybir.ActivationFunctionType.Sqrt,
                                 bias=eps_t[:], scale=1.0)
            rstd = work.tile([npc, 1], F32)
            nc.vector.reciprocal(out=rstd[:], in_=std[:])

            xn = work.tile([npc, DV], F32)
            for s in range(2):
                nc.vector.tensor_scalar(
                    out=xn[:, s * HD:(s + 1) * HD], in0=h_ps[s][:],
                    scalar1=mv[:, 0:1], scalar2=rstd[:],
                    op0=mybir.AluOpType.subtract, op1=mybir.AluOpType.mult)
            if not skip_ln_affine:
                nc.vector.tensor_mul(out=xn[:], in0=xn[:], in1=gb[:])
                nc.vector.tensor_add(out=xn[:], in0=xn[:], in1=bb[:])

            # ---- exact GELU (y_out path) ----
            y_sb = work.tile([npc, DV], F32)
            nc.scalar.activation(out=y_sb[:], in_=xn[:],
                                 func=mybir.ActivationFunctionType.Gelu)
            nc.sync.dma_start(y_d[:], y_sb[:])

            # ---- heads: transpose pre-GELU xn, gelu on transposed tiles ----
            lg_ps = ps_mm.tile([npc, NH], F32)
            for k in range(KD):
                psA = ps_t.tile([128, npc], F32, name=f"ypsA{k}", tag="psA")
                nc.tensor.transpose(
                    out=psA[:], in_=xn[:, k * 128:(k + 1) * 128],
                    identity=ident[0:npc, 0:npc])
                yt = work.tile([128, npc], F32, name=f"yt{k}", tag=f"yt{k}")
                nc.scalar.activation(out=yt[:], in_=psA[:],
                                     func=mybir.ActivationFunctionType.Gelu)
                nc.tensor.matmul(out=lg_ps[:], lhsT=yt[:], rhs=ht[:, k, :],
                                 start=(k == 0), stop=(k == KD - 1))
            lg_sb = work.tile([npc, NH], F32)
            nc.vector.tensor_copy(out=lg_sb[:], in_=lg_ps[:])
            nc.sync.dma_start(lg_d[:], lg_sb[:])

    nc.compile()
    return nc


def _weights_layout(W_ff, Wh, Wd):
    Wt = np.ascontiguousarray(W_ff.astype(np.float32).T)
    wt = np.ascontiguousarray(Wt.reshape(KD, 128, DV).transpose(1, 0, 2))
    H = np.concatenate([Wh.astype(np.float32).reshape(NUM_LABELS * MAX_ATT, DV),
                        Wd.astype(np.float32).reshape(1, DV)], axis=0)
    Ht = np.ascontiguousarray(H.T)
    ht = np.ascontiguousarray(Ht.reshape(KD, 128, NH).transpose(1, 0, 2))
    return wt, ht


def build_and_run(inputs, trace=False, trace_cores=None):
    x = np.asarray(inputs["x"], dtype=np.float32)
    boxes = np.asarray(inputs["boxes"], dtype=np.float32)
    box_labels = np.asarray(inputs["box_labels"], dtype=np.int32)
    W_ff = np.asarray(inputs["W_ff"], dtype=np.float32)
    b_ff = np.asarray(inputs["b_ff"], dtype=np.float32)
    ln_g = np.asarray(inputs["ln_g"], dtype=np.float32)
    ln_b = np.asarray(inputs["ln_b"], dtype=np.float32)
    Wh = np.asarray(inputs["Wh"], dtype=np.float32)
    bh = np.asarray(inputs["bh"], dtype=np.float32)
    Wd = np.asarray(inputs["Wd"], dtype=np.float32)
    bd = np.asarray(inputs["bd"], dtype=np.float32)

    skip_bff = bool(np.all(b_ff == 0.0))
    skip_ln_affine = bool(np.all(ln_g == 1.0) and np.all(ln_b == 0.0))

    cfg, packeds, orders = _preprocess(x, boxes)
    npc = cfg["npc"]
    nc = _build_program(cfg, skip_bff, skip_ln_affine)
    wt, ht = _weights_layout(W_ff, Wh, Wd)

    in_maps = []
    for cc in range(N_CORES):
        m = {"packed": packeds[cc], "wt": wt, "ht": ht}
        if not skip_bff:
            m["bff"] = np.ascontiguousarray(b_ff.reshape(1, DV))
        if not skip_ln_affine:
            m["lng"] = np.ascontiguousarray(ln_g.reshape(1, DV))
            m["lnb"] = np.ascontiguousarray(ln_b.reshape(1, DV))
        in_maps.append(m)

    kw = {}
    if trace:
        kw = dict(trace=True)
        if trace_cores is not None:
            kw["trace_cores"] = trace_cores
    res = bass_utils.run_bass_kernel_spmd(
        nc, in_maps, core_ids=list(range(N_CORES)), **kw)

    nbox = boxes.shape[0]
    y_full = np.zeros((nbox, DV), np.float32)
    logits_all = np.zeros((nbox, NH), np.float32)
    for cc in range(N_CORES):
        y_full[orders[cc]] = res.results[cc]["y_out"][:npc]
        logits_all[orders[cc]] = res.results[cc]["lg_out"][:npc]

    disr = logits_all[:, NH - 1:NH] + bd.reshape(1, 1)
    att = logits_all[:, :NUM_LABELS * MAX_ATT].reshape(
        nbox, NUM_LABELS, MAX_ATT)[np.arange(nbox), box_labels] + bh[box_labels]
    valid = np.arange(MAX_ATT)[None, :] < ID2CAT[box_labels][:, None]
    att = np.where(valid, att, np.float32(0.0)).astype(np.float32)
    return (y_full.astype(np.float32), att, disr.astype(np.float32)), res


def kernel(**inputs):
    boxes = np.asarray(inputs["boxes"], dtype=np.float32)
    nbox = boxes.shape[0]
    pad = (-nbox) % N_CORES
    if pad:
        inputs = dict(inputs)
        inputs["boxes"] = np.concatenate([boxes, np.tile(boxes[:1], (pad, 1))])
        inputs["box_labels"] = np.concatenate(
            [np.asarray(inputs["box_labels"], np.int32),
             np.zeros(pad, np.int32)])
    outs, _ = build_and_run(inputs, trace=False)
    if pad:
        outs = tuple(o[:nbox] for o in outs)
    return outs
